# revision 1
# baseline (speedup 1.0000x reference)
"""Trainium2 Bass kernel for AdaptiveLRLinearWithChannel (moe_routing).

Math: out[n] = x[n] @ reshape(U[idx[n]] @ V, [IN, OUT]) + bias[idx[n]]
  x: [256, 1024, 256] f32, U: [512, 60], V: [60, 65536], bias: [512, 1, 256]

Strategy (8 NeuronCores, data/expert parallel over the selected-channel dim):
  - Host (sharding/layout layer): shard the 256 selected channels 32 per
    core; gather the per-channel weights W = (U @ V)[idx] and bias rows by
    indices; lay W out as [i%128, channel, i//128, o] and x as
    [channel, IN, B] so the contraction dim (IN) lands on SBUF partitions.
    The low-rank weight synthesis is cheap preprocessing (2 GFLOP, ~6% of
    total FLOPs); the 34.4 GFLOP batched einsum runs on the device, which
    is what the kernel is memory-bound on (x in + out out = 67MB/core).
  - Device: per channel, per 128-row batch chunk: two accumulating fp32r
    matmuls (K=128 each) into PSUM, DVE bias-add into an SBUF staging
    tile, batched 512KB DMA to the output.
"""

import sys

for _p in ("/opt/trn_rl_repo",):
    if _p not in sys.path:
        sys.path.append(_p)

import ml_dtypes
import numpy as np

from concourse import bacc
import concourse.mybir as mybir
import concourse.bass_utils as bass_utils
from concourse.tile import TileContext

N_CORES = 8
N_SEL = 256
B = 1024
IN = 256
OUT = 256
RANK = 60

N_LOC = N_SEL // N_CORES          # 32 channels per core
K_CH = IN // 128                  # 2 i-chunks of 128
B_CH = B // 128                   # 8 batch chunks of 128
OG = 4                            # batch chunks per output staging group

F32 = mybir.dt.float32
F32R = mybir.dt.float32r

_NC_CACHE = None


def _build():
    nc = bacc.Bacc()
    BF16 = mybir.dt.bfloat16
    xt = nc.declare_dram_parameter("xt", [N_LOC, IN, B], F32, isOutput=False)
    # W shipped as a bf16 Dekker split (hi + lo ~= 16 mantissa bits) to halve
    # the weight-stream bytes with no accuracy loss vs fp32r compute.
    w2h = nc.declare_dram_parameter("w2h", [128, N_LOC, K_CH, OUT], BF16, isOutput=False)
    w2l = nc.declare_dram_parameter("w2l", [128, N_LOC, K_CH, OUT], BF16, isOutput=False)
    bias = nc.declare_dram_parameter("bias", [N_LOC, OUT], F32, isOutput=False)
    out = nc.declare_dram_parameter("out", [N_LOC, B, OUT], F32, isOutput=True)

    W2_GRP = 8  # channels per W2-load chunk (lets channel-0 compute start early)

    with TileContext(nc) as tc:
        with (
            tc.tile_pool(name="const", bufs=1) as cpool,
            tc.tile_pool(name="wstg", bufs=2) as wpool,
            tc.tile_pool(name="xp", bufs=4) as xpool,
            tc.tile_pool(name="bp", bufs=2) as bpool,
            tc.tile_pool(name="op", bufs=4) as opool,
            tc.tile_pool(name="psm", bufs=6, space="PSUM") as psmp,
        ):
            # W2[p, c, k, o] = W[c, k*128+p, o]; rhs slices are W2[:, c, k, :]
            W2 = cpool.tile([128, N_LOC, K_CH, OUT], F32R)
            for c0 in range(0, N_LOC, W2_GRP):
                hi_t = wpool.tile([128, W2_GRP, K_CH, OUT], BF16)
                lo_t = wpool.tile([128, W2_GRP, K_CH, OUT], BF16)
                nc.sync.dma_start(out=hi_t[:], in_=w2h[:, c0 : c0 + W2_GRP, :, :])
                nc.sync.dma_start(out=lo_t[:], in_=w2l[:, c0 : c0 + W2_GRP, :, :])
                nc.vector.tensor_add(W2[:, c0 : c0 + W2_GRP, :, :], hi_t[:], lo_t[:])
            # all 32 bias rows on partition 0; broadcast per channel via gpsimd
            brow = cpool.tile([1, N_LOC * OUT], F32)
            nc.sync.dma_start(
                out=brow[:], in_=bias[:].rearrange("c o -> (c o)").unsqueeze(0)
            )

            for c in range(N_LOC):
                xs = xpool.tile([128, K_CH, B], F32R)
                nc.sync.dma_start(
                    out=xs[:],
                    in_=xt[c].rearrange("(k p) b -> p k b", p=128).bitcast(F32R),
                )
                bb = bpool.tile([128, OUT], F32)
                nc.gpsimd.partition_broadcast(bb[:], brow[0:1, c * OUT : (c + 1) * OUT])
                for g in range(B_CH // OG):
                    osb = opool.tile([128, OG, OUT], F32)
                    for j in range(OG):
                        bk = g * OG + j
                        po = psmp.tile([128, OUT], F32)
                        nc.tensor.matmul(
                            po[:],
                            xs[:, 0, bk * 128 : (bk + 1) * 128],
                            W2[:, c, 0, :],
                            start=True,
                            stop=False,
                        )
                        nc.tensor.matmul(
                            po[:],
                            xs[:, 1, bk * 128 : (bk + 1) * 128],
                            W2[:, c, 1, :],
                            start=False,
                            stop=True,
                        )
                        nc.vector.tensor_add(osb[:, j, :], po[:], bb[:])
                    nc.scalar.dma_start(
                        out=out[c].rearrange("(g j p) o -> g p j o", p=128, j=OG)[g],
                        in_=osb[:],
                    )
    nc.finalize()
    return nc


def _get_nc():
    global _NC_CACHE
    if _NC_CACHE is None:
        _NC_CACHE = _build()
    return _NC_CACHE


def make_in_maps(x, indices, weights_U, weights_V, bias):
    x = np.asarray(x, dtype=np.float32)
    idx = np.asarray(indices).astype(np.int64)
    u = np.asarray(weights_U, dtype=np.float32)
    v = np.asarray(weights_V, dtype=np.float32)
    b = np.asarray(bias, dtype=np.float32)

    # Per-channel weight gather + low-rank synthesis (preprocessing).
    w_sel = (u[idx] @ v).reshape(N_SEL, K_CH, 128, OUT)  # [n, k, p, o]

    in_maps = []
    for core in range(N_CORES):
        s = slice(core * N_LOC, (core + 1) * N_LOC)
        ii = idx[s]
        w2 = np.ascontiguousarray(w_sel[s].transpose(2, 0, 1, 3))
        w2h = w2.astype(ml_dtypes.bfloat16)
        w2l = (w2 - w2h.astype(np.float32)).astype(ml_dtypes.bfloat16)
        in_maps.append(
            {
                "xt": np.ascontiguousarray(x[s].transpose(0, 2, 1)),
                "w2h": w2h,
                "w2l": w2l,
                "bias": np.ascontiguousarray(b[ii, 0, :]),
            }
        )
    return in_maps


def kernel(x, indices, weights_U, weights_V, bias):
    in_maps = make_in_maps(x, indices, weights_U, weights_V, bias)
    nc = _get_nc()
    res = bass_utils.run_bass_kernel_spmd(nc, in_maps, core_ids=list(range(N_CORES)))
    return np.concatenate([res.results[i]["out"] for i in range(N_CORES)], axis=0)



# revision 2
# speedup vs baseline: 2.1211x; 2.1211x over previous
"""Trainium2 Bass kernel for AdaptiveLRLinearWithChannel (moe_routing).

Math: out[n] = x[n] @ reshape(U[idx[n]] @ V, [IN, OUT]) + bias[idx[n]]
  x: [256, 1024, 256] f32, U: [512, 60], V: [60, 65536], bias: [512, 1, 256]

Strategy (8 NeuronCores, data/expert parallel over the selected-channel dim):
  - Host (sharding/layout layer): shard the 256 selected channels 32 per
    core; synthesize the per-channel weights W = (U @ V)[idx] (cheap, 2
    GFLOP) and convert x / W to bf16.  The rel-err budget (2e-2) dwarfs
    bf16 quantization noise (~2e-3 measured end to end), and bf16 halves
    both HBM traffic and tensor-engine time vs fp32.
  - Device: per channel, per 128-row batch chunk: two accumulating bf16
    matmuls (K=128 each) into PSUM, then a PSUM->SBUF cast copy to bf16
    (alternating Vector/Scalar engines so neither is the bottleneck), and
    1MB batched DMAs in/out.  Bias is added on the host after readback,
    so the device does matmul + cast only.
  - All DRAM tensors are partition-major so every DMA moves 128 x 8KB
    contiguous lines (~line-rate on the 358 GB/s/core HBM interface).
    Per-core traffic: x 16.75MB in + W 4MB in + out 16.75MB out.
"""

import sys

for _p in ("/opt/trn_rl_repo",):
    if _p not in sys.path:
        sys.path.append(_p)

import ml_dtypes
import numpy as np

from concourse import bacc
import concourse.mybir as mybir
import concourse.bass_utils as bass_utils
from concourse.tile import TileContext

N_CORES = 8
N_SEL = 256
B = 1024
IN = 256
OUT = 256
RANK = 60

N_LOC = N_SEL // N_CORES          # 32 channels per core
K_CH = IN // 128                  # 2 contraction chunks of 128
B_CH = B // 128                   # 8 batch chunks of 128
PAIR = 2                          # channels per x/out DMA (1MB transfers)
W_GRP = 8                         # channels per W chunk load (1MB)

F32 = mybir.dt.float32
BF16 = mybir.dt.bfloat16

_NC_CACHE = None


def _build():
    nc = bacc.Bacc()
    # xt[p, c, k, b] = x[c, b, k*128+p] ; w2[p, c, k, o] = W[c, k*128+p, o]
    xt = nc.declare_dram_parameter("xt", [128, N_LOC, K_CH, B], BF16, isOutput=False)
    w2 = nc.declare_dram_parameter("w2", [128, N_LOC, K_CH, OUT], BF16, isOutput=False)
    # out[p, c, bk, o] = y[c, bk*128+p, o] (pre-bias, bf16)
    out = nc.declare_dram_parameter("out", [128, N_LOC, B_CH, OUT], BF16, isOutput=True)

    with TileContext(nc) as tc:
        with (
            tc.tile_pool(name="wp", bufs=1) as wpool,
            tc.tile_pool(name="xp", bufs=3) as xpool,
            tc.tile_pool(name="op", bufs=3) as opool,
            tc.tile_pool(name="ps", bufs=8, space="PSUM") as psmp,
        ):
            W2 = wpool.tile([128, N_LOC, K_CH, OUT], BF16)
            # Interleave the first x loads with the W chunks so channel-0
            # compute starts after ~2MB of DMA instead of ~5MB.
            xtiles = {}

            def load_pair(c0):
                xs = xpool.tile([128, PAIR, K_CH, B], BF16)
                nc.sync.dma_start(out=xs[:], in_=xt[:, c0 : c0 + PAIR, :, :])
                xtiles[c0] = xs

            load_pair(0)
            for i, c0 in enumerate(range(0, N_LOC, W_GRP)):
                nc.sync.dma_start(
                    out=W2[:, c0 : c0 + W_GRP, :, :],
                    in_=w2[:, c0 : c0 + W_GRP, :, :],
                )
                if i + 1 < N_LOC // PAIR:
                    load_pair((i + 1) * PAIR)

            for pi, c0 in enumerate(range(0, N_LOC, PAIR)):
                if c0 not in xtiles:
                    load_pair(c0)
                xs = xtiles.pop(c0)
                osb = opool.tile([128, PAIR, B_CH, OUT], BF16)
                for ci in range(PAIR):
                    c = c0 + ci
                    for h in range(B_CH // 2):
                        po = psmp.tile([128, 2, OUT], F32)  # one full PSUM bank
                        for j in range(2):
                            bk = h * 2 + j
                            nc.tensor.matmul(
                                po[:, j, :],
                                xs[:, ci, 0, bk * 128 : (bk + 1) * 128],
                                W2[:, c, 0, :],
                                start=True,
                                stop=False,
                            )
                            nc.tensor.matmul(
                                po[:, j, :],
                                xs[:, ci, 1, bk * 128 : (bk + 1) * 128],
                                W2[:, c, 1, :],
                                start=False,
                                stop=True,
                            )
                        dst = osb[:, ci, h * 2 : h * 2 + 2, :]
                        if h % 2 == 0:
                            nc.vector.tensor_copy(dst, po[:])
                        else:
                            nc.scalar.copy(dst, po[:])
                nc.scalar.dma_start(out=out[:, c0 : c0 + PAIR, :, :], in_=osb[:])
    nc.finalize()
    return nc


def _get_nc():
    global _NC_CACHE
    if _NC_CACHE is None:
        _NC_CACHE = _build()
    return _NC_CACHE


def make_in_maps(x, indices, weights_U, weights_V, bias):
    x = np.asarray(x, dtype=np.float32)
    idx = np.asarray(indices).astype(np.int64)
    u = np.asarray(weights_U, dtype=np.float32)
    v = np.asarray(weights_V, dtype=np.float32)

    # Per-channel weight gather + low-rank synthesis (host preprocessing).
    w_sel = (u[idx] @ v).reshape(N_SEL, K_CH, 128, OUT)  # [n, k, p, o]

    in_maps = []
    for core in range(N_CORES):
        s = slice(core * N_LOC, (core + 1) * N_LOC)
        xtc = x[s].reshape(N_LOC, B, K_CH, 128).transpose(3, 0, 2, 1)
        w2c = w_sel[s].transpose(2, 0, 1, 3)
        in_maps.append(
            {
                "xt": np.ascontiguousarray(xtc).astype(ml_dtypes.bfloat16),
                "w2": np.ascontiguousarray(w2c).astype(ml_dtypes.bfloat16),
            }
        )
    return in_maps


def gather_output(results, indices, bias):
    idx = np.asarray(indices).astype(np.int64)
    b = np.asarray(bias, dtype=np.float32)
    outs = []
    for core in range(N_CORES):
        s = slice(core * N_LOC, (core + 1) * N_LOC)
        ot = np.asarray(results[core]["out"])  # [128, N_LOC, B_CH, OUT] bf16
        o = ot.astype(np.float32).transpose(1, 2, 0, 3).reshape(N_LOC, B, OUT)
        o += b[idx[s]]  # [N_LOC, 1, OUT] broadcast over B
        outs.append(o)
    return np.concatenate(outs, axis=0)


def kernel(x, indices, weights_U, weights_V, bias):
    in_maps = make_in_maps(x, indices, weights_U, weights_V, bias)
    nc = _get_nc()
    res = bass_utils.run_bass_kernel_spmd(nc, in_maps, core_ids=list(range(N_CORES)))
    return gather_output(res.results, indices, bias)
